# revision 49
# baseline (speedup 1.0000x reference)
"""Trainium2 Bass kernel for nn_Autotuner_FFN (dense MLP, 8-core data parallel).

Strategy (fast path, used when g==1, be==0, bc2==0 — true for this model):
  * Host folds embeddings / 57 op-linears / log2 scalings and the LayerNorm
    mean-centering into one effective first-layer matrix W1_eff [185,1024],
    with the layer-1 bias as an extra constant-1 feature row.  Mean-centered
    rows make mean(P1)==0, so LN reduces to P = P * rsqrt(mean(P^2)+eps).
  * KEY ALGEBRA: the rsqrt scale s is per-SAMPLE (free dim) and positive, so
    it commutes through relu and through the next GEMM:
        relu(s∘P) = s∘relu(P);   (s∘A) @ W = s∘(A @ W).
    Device never broadcasts s over the hidden dim: layer-2 runs on raw
    relu(P1); the scales enter only via [1,CH] row math, division-free:
        e1 = v1+eps;  D = sqrt(sc2*pstat2 + eps*e1);  y = pst3/D + b3.
  * The sign(x)*ln(|x|+1) feature transform is applied on the host inside
    XT, so ScalarE uses only {Sqrt, Relu, Square, Identity} — one act
    table set, zero LoadActFuncSet switches.
  * Each PSUM m-tile is drained twice (HW allows one PSUM operand per
    DVE op): relu on DVE (tensor_scalar max) -> next-layer input, square
    on ScalarE -> LN stats; different engines pipeline across banks.
  * LN statistics GEMMs run in fp8 DoubleRow (squares e5m2, per-hidden
    1/(H*c^2) weights e4m3 padded to M=32, K=256/instr).  fp8 DoubleRow
    layer 2 is implemented (l2_fp8) but off: it breaks the 2e-2 gate.
  * Per-hidden scales c1[h], c2[j] (chosen from a host-side activation
    sample) are folded into W1/W2/W3/stat-weights so device tensors sit in
    a quantization-friendly range; all folds are exact in infinite
    precision.
  * Batch 65536 is sharded 8192/core across 8 NeuronCores (pure DP).
"""
import numpy as np

import concourse.bass as bass
import concourse.tile as tile
from concourse import bacc, mybir
from concourse.bass_utils import run_bass_kernel_spmd

AF = mybir.ActivationFunctionType
ALU = mybir.AluOpType
PM = mybir.MatmulPerfMode
F32 = mybir.dt.float32
F16 = mybir.dt.float16
E4 = mybir.dt.float8e4
E5 = mybir.dt.float8e5

B = 65536
N_CORES = 8
B_CORE = B // N_CORES          # 8192
CH = 512                       # batch chunk (one PSUM bank wide)
NCH = B_CORE // CH             # 16
HID = 1024
MT = HID // 128                # 8 hidden m-tiles
KA, KC = 128, 57               # feature K tiles (125+bias+2pad | 57 transformed)
EPS = 1e-5
LN2 = float(np.log(2.0))
T1 = 4.0                       # target std of scaled layer-1 preacts
T2 = 16.0                      # target std of scaled layer-2 preacts


# ---------------------------------------------------------------- host folds
def _fold_weights(inp):
    f8 = lambda x: np.asarray(x, np.float64)
    W1 = f8(inp["W1"]); b1 = f8(inp["b1"])
    emb_kc = f8(inp["emb_kc"]); emb_nl = f8(inp["emb_nl"])
    op_W = f8(inp["op_W"]); op_b = f8(inp["op_b"])
    emb_c = f8(inp["emb_contig"]); emb_s = f8(inp["emb_scalar"])
    emb_i = f8(inp["emb_indirect"])
    H = W1.shape[1]
    rows_A = []
    bias = b1.copy()
    rows_A.append(emb_kc @ W1[0:16])
    rows_A.append(emb_nl @ W1[16:32])
    W1_op = W1[32:944].reshape(57, 16, H)
    rows_A.append(np.einsum("ij,ijh->ih", op_W, W1_op))
    bias += np.einsum("ij,ijh->h", op_b, W1_op)
    rd_f2, rd_bool, rd_ss = [], [], []
    wd_f2, wd_bool, wd_ss = [], [], []
    for base, f2l, booll, ssl in ((947, rd_f2, rd_bool, rd_ss),
                                  (1027, wd_f2, wd_bool, wd_ss)):
        for d in range(4):
            Wd = W1[base + 20 * d: base + 20 * d + 20]
            f2l.append(Wd[0:2])
            ssl.append(Wd[2:8] / LN2)
            rows_b = []
            for e, sl in ((emb_c, slice(8, 12)), (emb_s, slice(12, 16)),
                          (emb_i, slice(16, 20))):
                rows_b.append((e[1] - e[0]) @ Wd[sl])
                bias += e[0] @ Wd[sl]
            booll.append(np.stack(rows_b))
    rows_A += [np.concatenate(rd_f2), np.concatenate(rd_bool),
               np.concatenate(wd_f2), np.concatenate(wd_bool),
               W1[1110:1112]]
    A = np.concatenate(rows_A)
    C = np.concatenate([W1[944:947] / LN2, W1[1107:1110] / LN2,
                        W1[1112:1115] / LN2,
                        np.concatenate(rd_ss), np.concatenate(wd_ss)])
    W1_eff = np.concatenate([A, np.zeros((3, H)), C])       # [185, H]
    W1c = W1_eff - W1_eff.mean(axis=1, keepdims=True)
    bc1 = bias - bias.mean()
    W2 = f8(inp["W2"]); b2 = f8(inp["b2"])
    W2c = W2 - W2.mean(axis=1, keepdims=True)
    bc2 = b2 - b2.mean()
    return W1c, bc1, W2c, bc2


def _build_xt(inp, bias_row, host_ln):
    Bn = inp["op_vec"].shape[0]
    kc = np.asarray(inp["kernel_category_idx"]).astype(np.int64)
    nl = np.asarray(inp["num_of_loops_idx"]).astype(np.int64)
    f = lambda k: np.asarray(inp[k], np.float32)
    XT = np.zeros((KA + KC, Bn), np.float32)
    XT[0:10] = (np.arange(10)[:, None] == kc[None, :])
    XT[10:26] = (np.arange(16)[:, None] == nl[None, :])
    XT[26:83] = f("op_vec").T
    XT[83:91] = f("read_dep_float")[:, :, 0:2].reshape(Bn, 8).T
    XT[91:103] = np.asarray(inp["read_dep_bools"]).reshape(Bn, 12).T
    XT[103:111] = f("write_dep_float")[:, :, 0:2].reshape(Bn, 8).T
    XT[111:123] = np.asarray(inp["write_dep_bools"]).reshape(Bn, 12).T
    XT[123:125] = f("rest_vec")[:, 3:5].T
    if bias_row:
        XT[125] = 1.0
    XT[128:131] = f("size_hints").T
    XT[131:137] = f("rest_vec")[:, [0, 1, 2, 5, 6, 7]].T
    XT[137:161] = f("read_dep_float")[:, :, 2:8].reshape(Bn, 24).T
    XT[161:185] = f("write_dep_float")[:, :, 2:8].reshape(Bn, 24).T
    if host_ln:
        # fold the u = sign(x)*ln(|x|+1) feature transform into the input
        x = XT[128:185]
        XT[128:185] = np.sign(x) * np.log1p(np.abs(x))
    return XT


def _pack128(v, dtype=np.float32):
    """[1024] -> [128, 8] with v[m*128+p] at [p, m]."""
    return np.ascontiguousarray(
        np.asarray(v, np.float64).reshape(8, 128).T.astype(dtype))


def _to_f16(a):
    return np.asarray(a, np.float16)


def _to_e4(a):
    import ml_dtypes
    return np.asarray(np.clip(np.asarray(a, np.float64), -240.0, 240.0),
                      ml_dtypes.float8_e4m3fn)


# ---------------------------------------------------------------- device prog
def _dedupe_ldweights(nc):
    """Remove InstLdweights that reload the stationary already on the PE
    array (identical weights AP as the previous PE weight load, no sync).
    The paired non-self-loading matmults then reuse the loaded weights.
    LDW is not hidden by the engine on TRN2, so each removal saves the
    full reload time."""
    removed = 0
    for b in nc.m.functions[0].blocks:
        last_sig = None
        keep = []
        for i in b.instructions:
            if isinstance(i, mybir.InstLdweights):
                w = i.ins[0]
                sig = (w.memref, w.offset, str(w.ap), str(w.dtype),
                       i.perf_mode, i.is_transpose,
                       getattr(i, "tile_position", None))
                si = i.sync_info
                has_sync = si is not None and (
                    len(si.on_wait) > 0 or len(si.on_update) > 0)
                if sig == last_sig and not has_sync:
                    removed += 1
                    continue
                last_sig = sig
            elif isinstance(i, (mybir.InstMatmult, mybir.InstEventSemaphore)):
                pass  # does not change the loaded stationary
            elif getattr(i, "engine", None) == mybir.EngineType.PE:
                last_sig = None  # unknown PE instruction: be conservative
            keep.append(i)
        b.instructions[:] = keep
    return removed


DEFAULT_CFG = dict(
    l2_fp8=False,        # layer-2 GEMM in e4m3 DoubleRow (else f16)
    stats_fp8=True,      # LN-stats GEMMs in fp8 DoubleRow (else f16)
    pair_l1=True,        # drain layer-1 PSUM in 2-bank pairs
    pair_l2=True,        # drain layer-2 PSUM in 2-bank pairs
    ps_mm_bufs=2,        # [128,1024] 2-bank rotating matmul buffers
    ps_st_bufs=3,        # [1,512] stat banks
    a_bufs=2, s_bufs=2, xin_bufs=3, xr_bufs=2, small_bufs=3,
    nch=NCH,             # chunks per pass (reduce only for compile bisects)
    probe_mm_only=False,  # perf probe: matmuls only, constant activations
    probe_pairmm=False,   # perf probe: weight-shared matmul pairs in L2
    dedupe_ldw=False,     # post-compile removal of redundant weight loads
    chunk_pairs=False,    # process chunk pairs sharing each weight load
)


def build_program_fast(loop_iters=None, cfg=None):
    cfg = {**DEFAULT_CFG, **(cfg or {})}
    L2DT = E4 if cfg["l2_fp8"] else F16
    SQDT = E5 if cfg["stats_fp8"] else F16
    UDT = E4 if cfg["stats_fp8"] else F16
    nc = bacc.Bacc("TRN2", target_bir_lowering=False, debug=False)
    xt = nc.dram_tensor("xt", [KA + KC, B_CORE], F16, kind="ExternalInput")
    w1 = nc.dram_tensor("w1", [KA + KC, HID], F16, kind="ExternalInput")
    w2 = nc.dram_tensor("w2", [128, MT * HID], L2DT, kind="ExternalInput")
    w3p = nc.dram_tensor("w3p", [128, MT], F16, kind="ExternalInput")
    # stats stationaries padded to M=32 per k-subtile (DR needs >=32-wide
    # weight tiles); u sits in column 0 of each 32-block, rest zeros
    UW = 32 if cfg["stats_fp8"] else 1
    u1p = nc.dram_tensor("u1p", [128, MT * UW], UDT, kind="ExternalInput")
    u2p = nc.dram_tensor("u2p", [128, MT * UW], UDT, kind="ExternalInput")
    sc1 = nc.dram_tensor("sc1", [1, 1], F32, kind="ExternalInput")
    sc2 = nc.dram_tensor("sc2", [1, 1], F32, kind="ExternalInput")
    b3t = nc.dram_tensor("b3t", [1, 1], F32, kind="ExternalInput")
    y = nc.dram_tensor("y", [1, B_CORE], F32, kind="ExternalOutput")

    from contextlib import ExitStack
    with tile.TileContext(nc) as tc, ExitStack() as ctx, \
            nc.allow_low_precision(reason="fp16/fp8 rounding is intentional"):
        const = ctx.enter_context(tc.tile_pool(name="const", bufs=1))
        xin = ctx.enter_context(tc.tile_pool(name="xin", bufs=cfg["xin_bufs"]))
        bigA = ctx.enter_context(tc.tile_pool(name="bigA", bufs=cfg["a_bufs"]))
        bigS = ctx.enter_context(tc.tile_pool(name="bigS", bufs=cfg["s_bufs"]))
        small = ctx.enter_context(tc.tile_pool(name="small", bufs=cfg["small_bufs"]))
        ps_mm = ctx.enter_context(
            tc.tile_pool(name="ps_mm", bufs=cfg["ps_mm_bufs"], space="PSUM"))
        ps_st = ctx.enter_context(
            tc.tile_pool(name="ps_st", bufs=cfg["ps_st_bufs"], space="PSUM"))

        # ---- one-time setup: weights arrive pre-converted from host
        w1a = const.tile([128, HID], F16, tag="w1a")
        nc.sync.dma_start(w1a[:], w1.ap()[0:128, :])
        w1c = const.tile([KC, HID], F16, tag="w1c")
        nc.sync.dma_start(w1c[:], w1.ap()[128:185, :])
        w2s = const.tile([128, MT * HID], L2DT, tag="w2s")
        nc.sync.dma_start(w2s[:], w2.ap())
        w3s = const.tile([128, MT], F16, tag="w3s")
        nc.sync.dma_start(w3s[:], w3p.ap())
        u1s = const.tile([128, MT * UW], UDT, tag="u1s")
        nc.sync.dma_start(u1s[:], u1p.ap())
        u2s = const.tile([128, MT * UW], UDT, tag="u2s")
        nc.sync.dma_start(u2s[:], u2p.ap())
        sc1s = const.tile([1, 1], F32, tag="sc1s")
        nc.sync.dma_start(sc1s[:], sc1.ap())
        sc2s = const.tile([1, 1], F32, tag="sc2s")
        nc.sync.dma_start(sc2s[:], sc2.ap())
        b3s = const.tile([1, 1], F32, tag="b3s")
        nc.sync.dma_start(b3s[:], b3t.ap())
        eps_t = const.tile([1, 1], F32, tag="eps_t")
        nc.vector.memset(eps_t[:], EPS)

        def layer_mms_l2(q_out, m, a1r3, A1):
            """Accumulate layer-2 m-tile into q_out [128, CH]."""
            if cfg["l2_fp8"]:
                w2r3 = w2s[:].rearrange("p (k j) -> p k j", k=MT)
                for t in range(MT // 2):
                    nc.tensor.matmul(
                        q_out, w2r3[:, 2 * t:2 * t + 2, m * 128:(m + 1) * 128],
                        a1r3[:, 2 * t:2 * t + 2, :],
                        start=(t == 0), stop=(t == MT // 2 - 1),
                        perf_mode=PM.DoubleRow)
            else:
                for k in range(MT):
                    nc.tensor.matmul(
                        q_out,
                        w2s[:, k * HID + m * 128: k * HID + (m + 1) * 128],
                        A1[:, k * CH:(k + 1) * CH],
                        start=(k == 0), stop=(k == MT - 1))

        def stats_mms(pst, us, sr3, ts):
            """Emit stats accumulation for k-pair indices ts (subset of the
            group; caller splits the group around other matmuls so PE never
            waits on the last squares)."""
            NT = MT // 2 if cfg["stats_fp8"] else MT
            if cfg["stats_fp8"]:
                us3 = us[:].rearrange("p (k j) -> p k j", k=MT)
                for t in ts:
                    nc.tensor.matmul(pst, us3[:, 2 * t:2 * t + 2, :],
                                     sr3[:, 2 * t:2 * t + 2, :],
                                     start=(t == 0), stop=(t == NT - 1),
                                     perf_mode=PM.DoubleRow,
                                     skip_group_check=True)
            else:
                for t in ts:
                    nc.tensor.matmul(pst, us[:, t:t + 1], sr3[:, t, :],
                                     start=(t == 0), stop=(t == NT - 1),
                                     skip_group_check=True)

        def drain(pool_ps, tag, fill_one, relu_dst, sq_dst, paired):
            """Fill PSUM m-tiles via fill_one(psum_slice, m), drain with
            relu (DVE max) -> relu_dst and square (ScalarE) -> sq_dst.
            Each drain op reads PSUM exactly once (hardware limit)."""
            if paired:
                for i in range(MT // 2):
                    p = pool_ps.tile([128, 2 * CH], F32, tag=tag)
                    for sub in range(2):
                        fill_one(p[:, sub * CH:(sub + 1) * CH], 2 * i + sub)
                    sl = slice(2 * i * CH, (2 * i + 2) * CH)
                    nc.vector.tensor_scalar(out=relu_dst[:, sl], in0=p[:],
                                            scalar1=0.0, scalar2=None,
                                            op0=ALU.max)
                    nc.scalar.activation(sq_dst[:, sl], p[:], AF.Square)
            else:
                for m in range(MT):
                    p = pool_ps.tile([128, CH], F32, tag=tag)
                    fill_one(p[:], m)
                    sl = slice(m * CH, (m + 1) * CH)
                    nc.vector.tensor_scalar(out=relu_dst[:, sl], in0=p[:],
                                            scalar1=0.0, scalar2=None,
                                            op0=ALU.max)
                    nc.scalar.activation(sq_dst[:, sl], p[:], AF.Square)

        def chunk_l1(c):
            x1 = xin.tile([128, CH], F16, tag="x1")
            nc.sync.dma_start(x1[:], xt.ap()[0:128, c * CH:(c + 1) * CH])
            x2 = xin.tile([KC, CH], F16, tag="x2")
            nc.sync.dma_start(x2[:], xt.ap()[128:185, c * CH:(c + 1) * CH])

            # ---- layer 1: P1' in PSUM -> relu -> A1 (L2DT), square -> S1
            A1 = bigA.tile([128, MT * CH], L2DT, tag="A1")
            S1 = bigS.tile([128, MT * CH], SQDT, tag="S1")

            def fill1(pslice, m):
                nc.tensor.matmul(pslice, w1a[:, m * 128:(m + 1) * 128],
                                 x1[:], start=True, stop=False)
                nc.tensor.matmul(pslice, w1c[:, m * 128:(m + 1) * 128],
                                 x2[:], start=False, stop=True)

            drain(ps_mm, "pmm", fill1, A1, S1, cfg["pair_l1"])
            return A1, S1

        def chunk_rest(c, A1, S1):
            NT = MT // 2 if cfg["stats_fp8"] else MT
            pst1 = ps_st.tile([UW, CH], F32, tag="pst")
            s1r3 = S1[:].rearrange("p (k n) -> p k n", k=MT)
            stats_mms(pst1, u1s, s1r3, range(NT))

            # ---- layer 2: Q2' in PSUM -> relu -> A2 (f16), square -> S2
            A2 = bigA.tile([128, MT * CH], F16, tag="A2")
            S2 = bigS.tile([128, MT * CH], SQDT, tag="S2")
            a1r3 = A1[:].rearrange("p (k n) -> p k n", k=MT)

            def fill2(pslice, m):
                layer_mms_l2(pslice, m, a1r3, A1)

            drain(ps_mm, "pmm", fill2, A2, S2, cfg["pair_l2"])

            pst2 = ps_st.tile([UW, CH], F32, tag="pst")
            s2r3 = S2[:].rearrange("p (k n) -> p k n", k=MT)
            stats_mms(pst2, u2s, s2r3, range(NT))

            # ---- layer 3
            pst3 = ps_st.tile([1, CH], F32, tag="pst")
            for k in range(MT):
                nc.tensor.matmul(pst3[:], w3s[:, k:k + 1],
                                 A2[:, k * CH:(k + 1) * CH],
                                 start=(k == 0), stop=(k == MT - 1))
            return pst1, pst2, pst3

        def chunk_tail(c, state):
            # division-free LN-scale chain:
            #   e1  = v1 + eps = sc1*pstat1 + eps
            #   D^2 = e1*(v2+eps) = sc2*pstat2 + eps*e1
            #   y   = s1*s2*pst3 + b3 = pst3/D + b3
            # host supplies sc1e = eps*sc1 so e1e = eps*e1 comes in one op
            pst1, pst2, pst3 = state
            e1e = small.tile([1, CH], F32, tag="e1e")
            nc.vector.tensor_scalar(out=e1e[:], in0=pst1[0:1, :],
                                    scalar1=sc1s[:], scalar2=EPS * EPS,
                                    op0=ALU.mult, op1=ALU.add)
            q2s = small.tile([1, CH], F32, tag="q2s")
            nc.vector.tensor_scalar(out=q2s[:], in0=pst2[0:1, :],
                                    scalar1=sc2s[:], scalar2=None,
                                    op0=ALU.mult)
            dd = small.tile([1, CH], F32, tag="dd")
            nc.vector.tensor_add(dd[:], q2s[:], e1e[:])
            D = small.tile([1, CH], F32, tag="D")
            nc.scalar.activation(D[:], dd[:], AF.Sqrt)
            rD = small.tile([1, CH], F32, tag="rD")
            nc.vector.reciprocal(rD[:], D[:])
            yv = small.tile([1, CH], F32, tag="yv")
            nc.vector.tensor_mul(yv[:], rD[:], pst3[:])
            osb = small.tile([1, CH], F32, tag="osb")
            nc.scalar.activation(osb[:], yv[:], AF.Identity, bias=b3s[:])
            nc.sync.dma_start(y.ap()[0:1, c * CH:(c + 1) * CH], osb[:])

        def probe_chunk(c, A1c, A2c):
            x1 = xin.tile([128, CH], F16, tag="x1")
            nc.sync.dma_start(x1[:], xt.ap()[0:128, c * CH:(c + 1) * CH])
            x2 = xin.tile([KC, CH], F16, tag="x2")
            nc.sync.dma_start(x2[:], xt.ap()[128:185, c * CH:(c + 1) * CH])
            a1r3 = A1c[:].rearrange("p (k n) -> p k n", k=MT)
            for m in range(MT):
                p = ps_mm.tile([128, CH], F32, tag="pmm")
                nc.tensor.matmul(p[:], w1a[:, m * 128:(m + 1) * 128],
                                 x1[:], start=True, stop=False)
                nc.tensor.matmul(p[:], w1c[:, m * 128:(m + 1) * 128],
                                 x2[:], start=False, stop=True)
            if cfg["probe_pairmm"]:
                for m in range(MT):
                    p1 = ps_mm.tile([128, CH], F32, tag="pmm")
                    p2 = ps_mm.tile([128, CH], F32, tag="pmm")
                    for k in range(MT):
                        w = w2s[:, k * HID + m * 128: k * HID + (m + 1) * 128]
                        nc.tensor.matmul(p1[:], w, A1c[:, k * CH:(k + 1) * CH],
                                         start=(k == 0), stop=(k == MT - 1),
                                         skip_group_check=True)
                        nc.tensor.matmul(p2[:], w, A2c[:, k * CH:(k + 1) * CH],
                                         start=(k == 0), stop=(k == MT - 1),
                                         skip_group_check=True)
            else:
                for m in range(MT):
                    p = ps_mm.tile([128, CH], F32, tag="pmm")
                    layer_mms_l2(p[:], m, a1r3, A1c)
            pst3 = ps_st.tile([1, CH], F32, tag="pst")
            for k in range(MT):
                nc.tensor.matmul(pst3[:], w3s[:, k:k + 1],
                                 A2c[:, k * CH:(k + 1) * CH],
                                 start=(k == 0), stop=(k == MT - 1))
            osb = small.tile([1, CH], F32, tag="osb")
            nc.scalar.activation(osb[:], pst3[:], AF.Identity, bias=b3s[:])
            nc.sync.dma_start(y.ap()[0:1, c * CH:(c + 1) * CH], osb[:])

        def pair_l1(ca, cb):
            xs = []
            for c in (ca, cb):
                x1 = xin.tile([128, CH], F16, tag="x1")
                nc.sync.dma_start(x1[:], xt.ap()[0:128, c * CH:(c + 1) * CH])
                x2 = xin.tile([KC, CH], F16, tag="x2")
                nc.sync.dma_start(x2[:], xt.ap()[128:185, c * CH:(c + 1) * CH])
                xs.append((x1, x2))
            outs = []
            for _ in (ca, cb):
                A1 = bigA.tile([128, MT * CH], L2DT, tag="A1")
                S1 = bigS.tile([128, MT * CH], SQDT, tag="S1")
                outs.append((A1, S1))
            for m in range(MT):
                ps = [ps_mm.tile([128, CH], F32, tag="pmm", name=f"pl1_{i}")
                      for i in range(2)]
                for i in range(2):
                    nc.tensor.matmul(ps[i][:], w1a[:, m * 128:(m + 1) * 128],
                                     xs[i][0][:], start=True, stop=False,
                                     skip_group_check=True)
                for i in range(2):
                    nc.tensor.matmul(ps[i][:], w1c[:, m * 128:(m + 1) * 128],
                                     xs[i][1][:], start=False, stop=True,
                                     skip_group_check=True)
                for i in range(2):
                    sl = slice(m * CH, (m + 1) * CH)
                    nc.vector.tensor_scalar(out=outs[i][0][:, sl],
                                            in0=ps[i][:], scalar1=0.0,
                                            scalar2=None, op0=ALU.max)
                    nc.scalar.activation(outs[i][1][:, sl], ps[i][:],
                                         AF.Square)
            return outs

        def pair_stats(us, srs, tag_rows):
            NT = MT // 2 if cfg["stats_fp8"] else MT
            psts = [ps_st.tile([tag_rows, CH], F32, tag="pst", name=f"pstp_{i}")
                    for i in range(2)]
            if cfg["stats_fp8"]:
                us3 = us[:].rearrange("p (k j) -> p k j", k=MT)
                for t in range(NT):
                    for i in range(2):
                        nc.tensor.matmul(psts[i],
                                         us3[:, 2 * t:2 * t + 2, :],
                                         srs[i][:, 2 * t:2 * t + 2, :],
                                         start=(t == 0), stop=(t == NT - 1),
                                         perf_mode=PM.DoubleRow,
                                         skip_group_check=True)
            else:
                for t in range(NT):
                    for i in range(2):
                        nc.tensor.matmul(psts[i], us[:, t:t + 1],
                                         srs[i][:, t, :],
                                         start=(t == 0), stop=(t == NT - 1),
                                         skip_group_check=True)
            return psts

        def pair_rest(a1s1_pair):
            s1rs = [S1[:].rearrange("p (k n) -> p k n", k=MT)
                    for (_, S1) in a1s1_pair]
            pst1s = pair_stats(u1s, s1rs, UW)
            # read the stat banks immediately so the ring keeps flowing
            e1es = []
            for pst1 in pst1s:
                e1e = small.tile([1, CH], F32, tag="e1e")
                nc.vector.tensor_scalar(out=e1e[:], in0=pst1[0:1, :],
                                        scalar1=sc1s[:],
                                        scalar2=EPS * EPS,
                                        op0=ALU.mult, op1=ALU.add)
                e1es.append(e1e)

            a2s2 = []
            for _ in range(2):
                A2 = bigA.tile([128, MT * CH], F16, tag="A2")
                S2 = bigS.tile([128, MT * CH], SQDT, tag="S2")
                a2s2.append((A2, S2))
            a1r3s = [A1[:].rearrange("p (k n) -> p k n", k=MT)
                     for (A1, _) in a1s1_pair]
            for m in range(MT):
                qs = [ps_mm.tile([128, CH], F32, tag="pmm", name=f"pl2_{i}")
                      for i in range(2)]
                if cfg["l2_fp8"]:
                    w2r3 = w2s[:].rearrange("p (k j) -> p k j", k=MT)
                    for t in range(MT // 2):
                        for i in range(2):
                            nc.tensor.matmul(
                                qs[i][:],
                                w2r3[:, 2 * t:2 * t + 2,
                                     m * 128:(m + 1) * 128],
                                a1r3s[i][:, 2 * t:2 * t + 2, :],
                                start=(t == 0), stop=(t == MT // 2 - 1),
                                perf_mode=PM.DoubleRow,
                                skip_group_check=True)
                else:
                    for k in range(MT):
                        w = w2s[:, k * HID + m * 128: k * HID + (m + 1) * 128]
                        for i in range(2):
                            nc.tensor.matmul(
                                qs[i][:], w,
                                a1s1_pair[i][0][:, k * CH:(k + 1) * CH],
                                start=(k == 0), stop=(k == MT - 1),
                                skip_group_check=True)
                for i in range(2):
                    sl = slice(m * CH, (m + 1) * CH)
                    nc.vector.tensor_scalar(out=a2s2[i][0][:, sl],
                                            in0=qs[i][:], scalar1=0.0,
                                            scalar2=None, op0=ALU.max)
                    nc.scalar.activation(a2s2[i][1][:, sl], qs[i][:],
                                         AF.Square)

            s2rs = [S2[:].rearrange("p (k n) -> p k n", k=MT)
                    for (_, S2) in a2s2]
            pst2s = pair_stats(u2s, s2rs, UW)
            q2ss = []
            for pst2 in pst2s:
                q2s = small.tile([1, CH], F32, tag="q2s")
                nc.vector.tensor_scalar(out=q2s[:], in0=pst2[0:1, :],
                                        scalar1=sc2s[:], scalar2=None,
                                        op0=ALU.mult)
                q2ss.append(q2s)

            pst3s = [ps_st.tile([1, CH], F32, tag="pst", name=f"pst3_{i}")
                     for i in range(2)]
            for k in range(MT):
                for i in range(2):
                    nc.tensor.matmul(pst3s[i][:], w3s[:, k:k + 1],
                                     a2s2[i][0][:, k * CH:(k + 1) * CH],
                                     start=(k == 0), stop=(k == MT - 1),
                                     skip_group_check=True)
            return [(e1es[i], q2ss[i], pst3s[i]) for i in range(2)]

        def pair_tail(c, state):
            e1e, q2s, pst3 = state
            dd = small.tile([1, CH], F32, tag="dd")
            nc.vector.tensor_add(dd[:], q2s[:], e1e[:])
            D = small.tile([1, CH], F32, tag="D")
            nc.scalar.activation(D[:], dd[:], AF.Sqrt)
            rD = small.tile([1, CH], F32, tag="rD")
            nc.vector.reciprocal(rD[:], D[:])
            yv = small.tile([1, CH], F32, tag="yv")
            nc.vector.tensor_mul(yv[:], rD[:], pst3[:])
            osb = small.tile([1, CH], F32, tag="osb")
            nc.scalar.activation(osb[:], yv[:], AF.Identity, bias=b3s[:])
            nc.sync.dma_start(y.ap()[0:1, c * CH:(c + 1) * CH], osb[:])

        def whole_pass_pairs():
            states = None
            for p in range(cfg["nch"] // 2):
                ca, cb = 2 * p, 2 * p + 1
                a1s1 = pair_l1(ca, cb)
                if states is not None:
                    pair_tail(ca - 2, states[0])
                    pair_tail(cb - 2, states[1])
                states = pair_rest(a1s1)
            pair_tail(cfg["nch"] - 2, states[0])
            pair_tail(cfg["nch"] - 1, states[1])

        def whole_pass():
            if cfg["chunk_pairs"]:
                whole_pass_pairs()
                return
            if cfg["probe_mm_only"]:
                A1c = const.tile([128, MT * CH], L2DT, tag="A1c")
                nc.vector.memset(A1c[:], 0.25)
                A2c = const.tile([128, MT * CH], F16, tag="A2c")
                nc.vector.memset(A2c[:], 0.25)
                for c in range(cfg["nch"]):
                    probe_chunk(c, A1c, A2c)
                return
            state = None
            for c in range(cfg["nch"]):
                a1s1 = chunk_l1(c)
                if state is not None:
                    chunk_tail(c - 1, state)
                state = chunk_rest(c, *a1s1)
            chunk_tail(cfg["nch"] - 1, state)

        if loop_iters is None:
            whole_pass()
        else:
            with tc.For_i(0, loop_iters, 1):
                whole_pass()
    nc.compile()
    if cfg["dedupe_ldw"]:
        _dedupe_ldweights(nc)
    return nc


def build_program(simple, loop_iters=None, cfg=None):
    if simple:
        return build_program_fast(loop_iters=loop_iters, cfg=cfg)
    return build_program_general(loop_iters=loop_iters)


# ------------------------------------------------------- general path (slow)
def build_program_general(loop_iters=None):
    """Fallback for nonzero be/bc2 or g != 1: baseline broadcast-LN design,
    correct for arbitrary affine parameters."""
    F32R = F16
    nc = bacc.Bacc("TRN2", target_bir_lowering=False, debug=False)
    xt = nc.dram_tensor("xt", [KA + KC, B_CORE], F32, kind="ExternalInput")
    w1 = nc.dram_tensor("w1", [KA + KC, HID], F32, kind="ExternalInput")
    w2 = nc.dram_tensor("w2", [HID, HID], F32, kind="ExternalInput")
    w3p = nc.dram_tensor("w3p", [128, MT], F32, kind="ExternalInput")
    bc1p = nc.dram_tensor("bc1p", [128, MT], F32, kind="ExternalInput")
    bc2p = nc.dram_tensor("bc2p", [128, MT], F32, kind="ExternalInput")
    g1p = nc.dram_tensor("g1p", [128, MT], F32, kind="ExternalInput")
    be1p = nc.dram_tensor("be1p", [128, MT], F32, kind="ExternalInput")
    g2p = nc.dram_tensor("g2p", [128, MT], F32, kind="ExternalInput")
    be2p = nc.dram_tensor("be2p", [128, MT], F32, kind="ExternalInput")
    b3t = nc.dram_tensor("b3t", [1, 1], F32, kind="ExternalInput")
    y = nc.dram_tensor("y", [1, B_CORE], F32, kind="ExternalOutput")

    from contextlib import ExitStack
    with tile.TileContext(nc) as tc, ExitStack() as ctx, \
            nc.allow_low_precision(reason="f16 rounding is intentional"):
        const = ctx.enter_context(tc.tile_pool(name="const", bufs=1))
        wstage = ctx.enter_context(tc.tile_pool(name="wstage", bufs=2))
        xin = ctx.enter_context(tc.tile_pool(name="xin", bufs=3))
        xr = ctx.enter_context(tc.tile_pool(name="xr", bufs=2))
        bigH = ctx.enter_context(tc.tile_pool(name="bigH", bufs=2))
        bigS = ctx.enter_context(tc.tile_pool(name="bigS", bufs=1))
        bigR1 = ctx.enter_context(tc.tile_pool(name="bigR1", bufs=1))
        bigR2 = ctx.enter_context(tc.tile_pool(name="bigR2", bufs=1))
        small = ctx.enter_context(tc.tile_pool(name="small", bufs=2))
        ps_mm = ctx.enter_context(tc.tile_pool(name="ps_mm", bufs=4, space="PSUM"))
        ps_st = ctx.enter_context(tc.tile_pool(name="ps_st", bufs=2, space="PSUM"))
        ps_vec = ctx.enter_context(tc.tile_pool(name="ps_vec", bufs=2, space="PSUM"))

        w1a_r = const.tile([128, HID], F32R, tag="w1a")
        st = wstage.tile([128, HID], F32, tag="stage")
        nc.sync.dma_start(st[:], w1.ap()[0:128, :])
        nc.vector.tensor_copy(w1a_r[:], st[:])
        w1c_r = const.tile([KC, HID], F32R, tag="w1c")
        stc = wstage.tile([KC, HID], F32, tag="stagec")
        nc.sync.dma_start(stc[:], w1.ap()[128:185, :])
        nc.vector.tensor_copy(w1c_r[:], stc[:])
        w2r = []
        for k in range(MT):
            stk = wstage.tile([128, HID], F32, tag="stage")
            nc.sync.dma_start(stk[:], w2.ap()[k * 128:(k + 1) * 128, :])
            t = const.tile([128, HID], F32R, tag=f"w2r{k}")
            nc.vector.tensor_copy(t[:], stk[:])
            w2r.append(t)
        w3p_r = const.tile([128, MT], F32R, tag="w3p")
        st3 = wstage.tile([128, MT], F32, tag="stages")
        nc.sync.dma_start(st3[:], w3p.ap())
        nc.vector.tensor_copy(w3p_r[:], st3[:])

        def load_small(name, dram):
            t = const.tile([128, MT], F32, tag=name)
            nc.sync.dma_start(t[:], dram.ap())
            return t
        bc1s = load_small("bc1s", bc1p); bc2s = load_small("bc2s", bc2p)
        g1s = load_small("g1s", g1p); be1s = load_small("be1s", be1p)
        g2s = load_small("g2s", g2p); be2s = load_small("be2s", be2p)
        b3s = const.tile([1, 1], F32, tag="b3s")
        nc.sync.dma_start(b3s[:], b3t.ap())
        ones_st = const.tile([128, 1], F32, tag="ones_st")
        nc.vector.memset(ones_st[:], 1.0)
        ones_col = const.tile([128, 1], F32R, tag="ones_col")
        nc.vector.tensor_copy(ones_col[:], ones_st[:])
        ones_rst = const.tile([1, 128], F32, tag="ones_rst")
        nc.vector.memset(ones_rst[:], 1.0)
        ones_row = const.tile([1, 128], F32R, tag="ones_row")
        nc.vector.tensor_copy(ones_row[:], ones_rst[:])
        eps_t = const.tile([1, 1], F32, tag="eps_t")
        nc.vector.memset(eps_t[:], EPS)

        def layer_norm_relu(Hb, g_s, be_s, out_pool, out_tag):
            sqb = bigS.tile([128, MT * CH], F32R, tag="sq")
            for m in range(MT):
                sl = slice(m * CH, (m + 1) * CH)
                nc.vector.tensor_mul(sqb[:, sl], Hb[:, sl], Hb[:, sl])
            pst = ps_st.tile([1, CH], F32, tag="pst")
            for m in range(MT):
                nc.tensor.matmul(pst[:], ones_col[:],
                                 sqb[:, m * CH:(m + 1) * CH],
                                 start=(m == 0), stop=(m == MT - 1))
            sd = small.tile([1, CH], F32, tag="sd")
            nc.scalar.activation(sd[:], pst[:], AF.Sqrt,
                                 bias=eps_t[:], scale=1.0 / HID)
            rs = small.tile([1, CH], F32R, tag="rs")
            nc.vector.reciprocal(rs[:], sd[:])
            pv = ps_vec.tile([128, CH], F32, tag="pv")
            nc.tensor.matmul(pv[:], ones_row[:], rs[:], start=True, stop=True)
            Rb = out_pool.tile([128, MT * CH], F32R, tag=out_tag)
            for m in range(MT):
                sl = slice(m * CH, (m + 1) * CH)
                nc.vector.tensor_mul(Hb[:, sl], Hb[:, sl], pv[:])
                nc.scalar.activation(Rb[:, sl], Hb[:, sl], AF.Relu,
                                     bias=be_s[:, m:m + 1],
                                     scale=g_s[:, m:m + 1])
            return Rb

        def chunk_body(c):
            x1 = xin.tile([128, CH], F32, tag="x1")
            nc.sync.dma_start(x1[:], xt.ap()[0:128, c * CH:(c + 1) * CH])
            x2 = xin.tile([KC, CH], F32, tag="x2")
            nc.sync.dma_start(x2[:], xt.ap()[128:185, c * CH:(c + 1) * CH])
            x1r = xr.tile([128, CH], F32R, tag="x1r")
            nc.vector.tensor_copy(x1r[:], x1[:])
            xab = xr.tile([KC, CH], F32, tag="xab")
            nc.vector.tensor_scalar(
                out=xab[:].bitcast(mybir.dt.int32),
                in0=x2[:].bitcast(mybir.dt.int32),
                scalar1=0x7FFFFFFF, scalar2=None, op0=ALU.bitwise_and)
            xln = xr.tile([KC, CH], F32, tag="xln")
            nc.scalar.activation(xln[:], xab[:], AF.Ln, bias=1.0)
            xsg = xr.tile([KC, CH], F32, tag="xsg")
            nc.scalar.activation(xsg[:], x2[:], AF.Sign)
            x2r = xr.tile([KC, CH], F32R, tag="x2r")
            nc.vector.tensor_mul(x2r[:], xsg[:], xln[:])

            H1 = bigH.tile([128, MT * CH], F32, tag="H")
            for m in range(MT):
                p1 = ps_mm.tile([128, CH], F32, tag="pmm")
                nc.tensor.matmul(p1[:], w1a_r[:, m * 128:(m + 1) * 128],
                                 x1r[:], start=True, stop=False)
                nc.tensor.matmul(p1[:], w1c_r[:, m * 128:(m + 1) * 128],
                                 x2r[:], start=False, stop=True)
                nc.scalar.activation(H1[:, m * CH:(m + 1) * CH], p1[:],
                                     AF.Identity, bias=bc1s[:, m:m + 1])
            R1 = layer_norm_relu(H1, g1s, be1s, bigR1, "R1")

            H2 = bigH.tile([128, MT * CH], F32, tag="H")
            for m in range(MT):
                p2 = ps_mm.tile([128, CH], F32, tag="pmm")
                for k in range(MT):
                    nc.tensor.matmul(p2[:], w2r[k][:, m * 128:(m + 1) * 128],
                                     R1[:, k * CH:(k + 1) * CH],
                                     start=(k == 0), stop=(k == MT - 1))
                nc.scalar.activation(H2[:, m * CH:(m + 1) * CH], p2[:],
                                     AF.Identity, bias=bc2s[:, m:m + 1])
            R2 = layer_norm_relu(H2, g2s, be2s, bigR2, "R2")

            p3 = ps_st.tile([1, CH], F32, tag="pst")
            for k in range(MT):
                nc.tensor.matmul(p3[:], w3p_r[:, k:k + 1],
                                 R2[:, k * CH:(k + 1) * CH],
                                 start=(k == 0), stop=(k == MT - 1))
            osb = small.tile([1, CH], F32, tag="osb")
            nc.scalar.activation(osb[:], p3[:], AF.Identity, bias=b3s[:])
            nc.sync.dma_start(y.ap()[0:1, c * CH:(c + 1) * CH], osb[:])

        if loop_iters is None:
            for c in range(NCH):
                chunk_body(c)
        else:
            with tc.For_i(0, loop_iters, 1):
                for c in range(NCH):
                    chunk_body(c)
    nc.compile()
    return nc


# ---------------------------------------------------------------- entry point
_CACHE = {}

# Measured-best on HW: 468-482us/pass.  The chunk-pair + LDW-dedupe mode
# (chunk_pairs=True, dedupe_ldw=True, a_bufs=3, s_bufs=3, ps_mm_bufs=4)
# removes 32 weight loads per chunk but its schedule measured slower
# (506us); kept available but off.
BEST_CFG = dict(DEFAULT_CFG, pair_l1=False, pair_l2=False, ps_mm_bufs=5)


def _get_program(simple):
    key = ("prog", simple)
    if key not in _CACHE:
        _CACHE[key] = build_program(simple, cfg=BEST_CFG if simple else None)
    return _CACHE[key]


def _make_fast_maps(inp, W1c, bc1, W2c, cfg):
    H = HID
    W1x = W1c.copy()
    W1x[125] = bc1                      # bias via constant-1 feature row
    XT = _build_xt(inp, bias_row=True, host_ln=True)

    # per-hidden scales from a strided data sample
    idx = np.arange(0, XT.shape[1], max(1, XT.shape[1] // 512))[:512]
    Xs = XT[:, idx].astype(np.float64)
    P1s = W1x.T @ Xs                    # [H, S]
    sd1 = P1s.std(axis=1) + 1e-9
    c1 = T1 / sd1
    A1s = np.maximum(P1s, 0.0)
    Q2s = W2c.T @ A1s
    sd2 = Q2s.std(axis=1) + 1e-9
    c2 = T2 / sd2

    W1s = _to_f16(W1x * c1[None, :])
    W2f = (W2c / c1[:, None]) * c2[None, :]          # [H, H]
    # pack [p, k*HID] with row h = k*128+p
    W2p = W2f.reshape(MT, 128, H).transpose(1, 0, 2).reshape(128, MT * H)
    w2dev = _to_e4(W2p) if cfg["l2_fp8"] else _to_f16(W2p)

    u1 = 1.0 / (H * c1 * c1)
    k1 = int(np.floor(np.log2(160.0 / u1.max())))
    u2 = 1.0 / (H * c2 * c2)
    k2 = int(np.floor(np.log2(160.0 / u2.max())))
    W3 = np.asarray(inp["W3"], np.float64)[:, 0]
    b3 = np.asarray(inp["b3"], np.float64)

    def upack(u):
        # [1024] -> [128, MT*UW]; value at [p, k*UW], zeros elsewhere
        if not cfg["stats_fp8"]:
            return _pack128(u, np.float16)
        UW = 32
        out = np.zeros((128, MT * UW), np.float64)
        out[:, ::UW] = _pack128(u, np.float64)
        return _to_e4(out)

    shared = {
        "w1": np.ascontiguousarray(W1s),
        "w2": np.ascontiguousarray(w2dev),
        "w3p": _pack128(W3 / c2, np.float16),
        "u1p": upack(u1 * 2.0 ** k1),
        "u2p": upack(u2 * 2.0 ** k2),
        "sc1": np.full((1, 1), EPS * 2.0 ** -k1, np.float32),
        "sc2": np.full((1, 1), 2.0 ** -k2, np.float32),
        "b3t": np.asarray(b3, np.float32).reshape(1, 1),
    }
    XT16 = _to_f16(XT)
    in_maps = []
    for c in range(N_CORES):
        m = dict(shared)
        m["xt"] = np.ascontiguousarray(XT16[:, c * B_CORE:(c + 1) * B_CORE])
        in_maps.append(m)
    return in_maps


def _make_general_maps(inp, W1c, bc1, W2c, bc2):
    XT = _build_xt(inp, bias_row=False, host_ln=False)
    g1 = np.asarray(inp["g1"], np.float32); be1 = np.asarray(inp["be1"], np.float32)
    g2 = np.asarray(inp["g2"], np.float32); be2 = np.asarray(inp["be2"], np.float32)
    W3 = np.asarray(inp["W3"], np.float32)
    b3 = np.asarray(inp["b3"], np.float32)
    shared = {
        "w1": W1c.astype(np.float32), "w2": W2c.astype(np.float32),
        "w3p": _pack128(W3[:, 0]),
        "bc1p": _pack128(bc1), "bc2p": _pack128(bc2),
        "g1p": _pack128(g1), "be1p": _pack128(be1),
        "g2p": _pack128(g2), "be2p": _pack128(be2),
        "b3t": b3.reshape(1, 1),
    }
    in_maps = []
    for c in range(N_CORES):
        m = dict(shared)
        m["xt"] = np.ascontiguousarray(
            XT[:, c * B_CORE:(c + 1) * B_CORE].astype(np.float32))
        in_maps.append(m)
    return in_maps


def make_in_maps(inputs, cfg=None):
    cfg = {**DEFAULT_CFG, **(cfg or BEST_CFG)}
    inp = {k: np.asarray(v) for k, v in inputs.items()}
    W1c, bc1, W2c, bc2 = _fold_weights(inp)
    g1 = np.asarray(inp["g1"]); be1 = np.asarray(inp["be1"])
    g2 = np.asarray(inp["g2"]); be2 = np.asarray(inp["be2"])
    simple = bool(
        np.all(g1 == 1.0) and np.all(g2 == 1.0)
        and np.all(be1 == 0.0) and np.all(be2 == 0.0)
        and np.all(np.abs(bc2) < 1e-12))
    if simple:
        return _make_fast_maps(inp, W1c, bc1, W2c, cfg), True
    return _make_general_maps(inp, W1c, bc1, W2c, bc2), False


def kernel(**inputs) -> np.ndarray:
    in_maps, simple = make_in_maps(inputs)
    nc = _get_program(simple)
    res = run_bass_kernel_spmd(nc, in_maps, core_ids=list(range(N_CORES)))
    y = np.concatenate([r["y"][0] for r in res.results])
    return y.reshape(B, 1).astype(np.float32)


if __name__ == "__main__":
    import jax
    import reference
    cpu = jax.devices("cpu")[0]
    with jax.default_device(cpu):
        inp = reference.setup_inputs()
        ref = np.asarray(reference.reference(**inp))
    out = kernel(**{k: np.asarray(v) for k, v in inp.items()})
    err = np.abs(out - ref)
    scale = np.abs(ref).max()
    print("max_abs", err.max(), "rel(vs scale)", err.max() / scale,
          "mean_rel", (err / (np.abs(ref) + 1e-6)).mean())


# revision 60
# speedup vs baseline: 1.2128x; 1.2128x over previous
"""Trainium2 Bass kernel for nn_Autotuner_FFN (dense MLP, 8-core data parallel).

Strategy (fast path, used when g==1, be==0, bc2==0 — true for this model):
  * Host folds embeddings / 57 op-linears / log2 scalings and the LayerNorm
    mean-centering into one effective first-layer matrix W1_eff [185,1024],
    with the layer-1 bias as an extra constant-1 feature row.  Mean-centered
    rows make mean(P1)==0, so LN reduces to P = P * rsqrt(mean(P^2)+eps).
  * KEY ALGEBRA: the rsqrt scale s is per-SAMPLE (free dim) and positive, so
    it commutes through relu and through the next GEMM:
        relu(s∘P) = s∘relu(P);   (s∘A) @ W = s∘(A @ W).
    Device never broadcasts s over the hidden dim: layer-2 runs on raw
    relu(P1); the scales enter only via [1,CH] row math, division-free:
        e1 = v1+eps;  D = sqrt(sc2*pstat2 + eps*e1);  y = pst3/D + b3.
  * The sign(x)*ln(|x|+1) feature transform is applied on the host inside
    XT, so ScalarE uses only {Sqrt, Relu, Square, Identity} — one act
    table set, zero LoadActFuncSet switches.
  * Each PSUM m-tile is drained twice (HW allows one PSUM operand per
    DVE op): relu on DVE (tensor_scalar max) -> next-layer input, square
    on ScalarE -> LN stats; different engines pipeline across banks.
  * Layer 3 never touches the TensorEngine: the layer-2 relu drain fuses
    the per-partition W3 scale (tensor_scalar max+mult), and the idle
    GpSimd engine finishes the dot product with a ping-pong tensor_add
    chain + partition_all_reduce — removing 8 matmuls and 8 unhidden
    ~56ns weight loads per chunk (HW-measured 442 -> 426.5us).
  * LN statistics GEMMs run in fp8 DoubleRow (squares e5m2, per-hidden
    1/(H*c^2) weights e4m3 padded to M=32, K=256/instr).  fp8 DoubleRow
    layer 2 is implemented (l2_fp8) but off: it breaks the 2e-2 gate.
  * Per-hidden scales c1[h], c2[j] (chosen from a host-side activation
    sample) are folded into W1/W2/W3/stat-weights so device tensors sit in
    a quantization-friendly range; all folds are exact in infinite
    precision.
  * Batch 65536 is sharded 8192/core across 8 NeuronCores (pure DP).
"""
import numpy as np

import concourse.bass as bass
import concourse.tile as tile
from concourse import bacc, mybir
from concourse.bass_utils import run_bass_kernel_spmd

AF = mybir.ActivationFunctionType
ALU = mybir.AluOpType
PM = mybir.MatmulPerfMode
F32 = mybir.dt.float32
F16 = mybir.dt.float16
E4 = mybir.dt.float8e4
E5 = mybir.dt.float8e5

B = 65536
N_CORES = 8
B_CORE = B // N_CORES          # 8192
CH = 512                       # batch chunk (one PSUM bank wide)
NCH = B_CORE // CH             # 16
HID = 1024
MT = HID // 128                # 8 hidden m-tiles
KA, KC = 128, 57               # feature K tiles (125+bias+2pad | 57 transformed)
EPS = 1e-5
LN2 = float(np.log(2.0))
T1 = 4.0                       # target std of scaled layer-1 preacts
T2 = 16.0                      # target std of scaled layer-2 preacts


# ---------------------------------------------------------------- host folds
def _fold_weights(inp):
    f8 = lambda x: np.asarray(x, np.float64)
    W1 = f8(inp["W1"]); b1 = f8(inp["b1"])
    emb_kc = f8(inp["emb_kc"]); emb_nl = f8(inp["emb_nl"])
    op_W = f8(inp["op_W"]); op_b = f8(inp["op_b"])
    emb_c = f8(inp["emb_contig"]); emb_s = f8(inp["emb_scalar"])
    emb_i = f8(inp["emb_indirect"])
    H = W1.shape[1]
    rows_A = []
    bias = b1.copy()
    rows_A.append(emb_kc @ W1[0:16])
    rows_A.append(emb_nl @ W1[16:32])
    W1_op = W1[32:944].reshape(57, 16, H)
    rows_A.append(np.einsum("ij,ijh->ih", op_W, W1_op))
    bias += np.einsum("ij,ijh->h", op_b, W1_op)
    rd_f2, rd_bool, rd_ss = [], [], []
    wd_f2, wd_bool, wd_ss = [], [], []
    for base, f2l, booll, ssl in ((947, rd_f2, rd_bool, rd_ss),
                                  (1027, wd_f2, wd_bool, wd_ss)):
        for d in range(4):
            Wd = W1[base + 20 * d: base + 20 * d + 20]
            f2l.append(Wd[0:2])
            ssl.append(Wd[2:8] / LN2)
            rows_b = []
            for e, sl in ((emb_c, slice(8, 12)), (emb_s, slice(12, 16)),
                          (emb_i, slice(16, 20))):
                rows_b.append((e[1] - e[0]) @ Wd[sl])
                bias += e[0] @ Wd[sl]
            booll.append(np.stack(rows_b))
    rows_A += [np.concatenate(rd_f2), np.concatenate(rd_bool),
               np.concatenate(wd_f2), np.concatenate(wd_bool),
               W1[1110:1112]]
    A = np.concatenate(rows_A)
    C = np.concatenate([W1[944:947] / LN2, W1[1107:1110] / LN2,
                        W1[1112:1115] / LN2,
                        np.concatenate(rd_ss), np.concatenate(wd_ss)])
    W1_eff = np.concatenate([A, np.zeros((3, H)), C])       # [185, H]
    W1c = W1_eff - W1_eff.mean(axis=1, keepdims=True)
    bc1 = bias - bias.mean()
    W2 = f8(inp["W2"]); b2 = f8(inp["b2"])
    W2c = W2 - W2.mean(axis=1, keepdims=True)
    bc2 = b2 - b2.mean()
    return W1c, bc1, W2c, bc2


def _build_xt(inp, bias_row, host_ln):
    Bn = inp["op_vec"].shape[0]
    kc = np.asarray(inp["kernel_category_idx"]).astype(np.int64)
    nl = np.asarray(inp["num_of_loops_idx"]).astype(np.int64)
    f = lambda k: np.asarray(inp[k], np.float32)
    XT = np.zeros((KA + KC, Bn), np.float32)
    XT[0:10] = (np.arange(10)[:, None] == kc[None, :])
    XT[10:26] = (np.arange(16)[:, None] == nl[None, :])
    XT[26:83] = f("op_vec").T
    XT[83:91] = f("read_dep_float")[:, :, 0:2].reshape(Bn, 8).T
    XT[91:103] = np.asarray(inp["read_dep_bools"]).reshape(Bn, 12).T
    XT[103:111] = f("write_dep_float")[:, :, 0:2].reshape(Bn, 8).T
    XT[111:123] = np.asarray(inp["write_dep_bools"]).reshape(Bn, 12).T
    XT[123:125] = f("rest_vec")[:, 3:5].T
    if bias_row:
        XT[125] = 1.0
    XT[128:131] = f("size_hints").T
    XT[131:137] = f("rest_vec")[:, [0, 1, 2, 5, 6, 7]].T
    XT[137:161] = f("read_dep_float")[:, :, 2:8].reshape(Bn, 24).T
    XT[161:185] = f("write_dep_float")[:, :, 2:8].reshape(Bn, 24).T
    if host_ln:
        # fold the u = sign(x)*ln(|x|+1) feature transform into the input
        x = XT[128:185]
        XT[128:185] = np.sign(x) * np.log1p(np.abs(x))
    return XT


def _pack128(v, dtype=np.float32):
    """[1024] -> [128, 8] with v[m*128+p] at [p, m]."""
    return np.ascontiguousarray(
        np.asarray(v, np.float64).reshape(8, 128).T.astype(dtype))


def _to_f16(a):
    return np.asarray(a, np.float16)


def _to_e4(a):
    import ml_dtypes
    return np.asarray(np.clip(np.asarray(a, np.float64), -240.0, 240.0),
                      ml_dtypes.float8_e4m3fn)


# ---------------------------------------------------------------- device prog
def _dedupe_ldweights(nc):
    """Remove InstLdweights that reload the stationary already on the PE
    array (identical weights AP as the previous PE weight load, no sync).
    The paired non-self-loading matmults then reuse the loaded weights.
    LDW is not hidden by the engine on TRN2, so each removal saves the
    full reload time."""
    removed = 0
    for b in nc.m.functions[0].blocks:
        last_sig = None
        keep = []
        for i in b.instructions:
            if isinstance(i, mybir.InstLdweights):
                w = i.ins[0]
                sig = (w.memref, w.offset, str(w.ap), str(w.dtype),
                       i.perf_mode, i.is_transpose,
                       getattr(i, "tile_position", None))
                si = i.sync_info
                has_sync = si is not None and (
                    len(si.on_wait) > 0 or len(si.on_update) > 0)
                if sig == last_sig and not has_sync:
                    removed += 1
                    continue
                last_sig = sig
            elif isinstance(i, (mybir.InstMatmult, mybir.InstEventSemaphore)):
                pass  # does not change the loaded stationary
            elif getattr(i, "engine", None) == mybir.EngineType.PE:
                last_sig = None  # unknown PE instruction: be conservative
            keep.append(i)
        b.instructions[:] = keep
    return removed


DEFAULT_CFG = dict(
    l2_fp8=False,        # layer-2 GEMM in e4m3 DoubleRow (else f16)
    stats_fp8=True,      # LN-stats GEMMs in fp8 DoubleRow (else f16)
    pair_l1=True,        # drain layer-1 PSUM in 2-bank pairs
    pair_l2=True,        # drain layer-2 PSUM in 2-bank pairs
    ps_mm_bufs=2,        # [128,1024] 2-bank rotating matmul buffers
    ps_st_bufs=3,        # [1,512] stat banks
    a_bufs=2, s_bufs=2, xin_bufs=3, xr_bufs=2, small_bufs=3,
    nch=NCH,             # chunks per pass (reduce only for compile bisects)
    probe_mm_only=False,  # perf probe: matmuls only, constant activations
    probe_pairmm=False,   # perf probe: weight-shared matmul pairs in L2
    dedupe_ldw=False,     # post-compile removal of redundant weight loads
    chunk_pairs=False,    # process chunk pairs sharing each weight load
    l3_gpsimd=False,      # layer-3 dot product on idle GpSimd instead of PE
)


def build_program_fast(loop_iters=None, cfg=None):
    cfg = {**DEFAULT_CFG, **(cfg or {})}
    L2DT = E4 if cfg["l2_fp8"] else F16
    SQDT = E5 if cfg["stats_fp8"] else F16
    UDT = E4 if cfg["stats_fp8"] else F16
    nc = bacc.Bacc("TRN2", target_bir_lowering=False, debug=False)
    xt = nc.dram_tensor("xt", [KA + KC, B_CORE], F16, kind="ExternalInput")
    w1 = nc.dram_tensor("w1", [KA + KC, HID], F16, kind="ExternalInput")
    w2 = nc.dram_tensor("w2", [128, MT * HID], L2DT, kind="ExternalInput")
    w3p = nc.dram_tensor("w3p", [128, MT], F16, kind="ExternalInput")
    # stats stationaries padded to M=32 per k-subtile (DR needs >=32-wide
    # weight tiles); u sits in column 0 of each 32-block, rest zeros
    UW = 32 if cfg["stats_fp8"] else 1
    u1p = nc.dram_tensor("u1p", [128, MT * UW], UDT, kind="ExternalInput")
    u2p = nc.dram_tensor("u2p", [128, MT * UW], UDT, kind="ExternalInput")
    sc1 = nc.dram_tensor("sc1", [1, 1], F32, kind="ExternalInput")
    sc2 = nc.dram_tensor("sc2", [1, 1], F32, kind="ExternalInput")
    b3t = nc.dram_tensor("b3t", [1, 1], F32, kind="ExternalInput")
    y = nc.dram_tensor("y", [1, B_CORE], F32, kind="ExternalOutput")

    from contextlib import ExitStack
    with tile.TileContext(nc) as tc, ExitStack() as ctx, \
            nc.allow_low_precision(reason="fp16/fp8 rounding is intentional"):
        const = ctx.enter_context(tc.tile_pool(name="const", bufs=1))
        xin = ctx.enter_context(tc.tile_pool(name="xin", bufs=cfg["xin_bufs"]))
        bigA = ctx.enter_context(tc.tile_pool(name="bigA", bufs=cfg["a_bufs"]))
        bigS = ctx.enter_context(tc.tile_pool(name="bigS", bufs=cfg["s_bufs"]))
        small = ctx.enter_context(tc.tile_pool(name="small", bufs=cfg["small_bufs"]))
        ps_mm = ctx.enter_context(
            tc.tile_pool(name="ps_mm", bufs=cfg["ps_mm_bufs"], space="PSUM"))
        ps_st = ctx.enter_context(
            tc.tile_pool(name="ps_st", bufs=cfg["ps_st_bufs"], space="PSUM"))

        # ---- one-time setup: weights arrive pre-converted from host
        w1a = const.tile([128, HID], F16, tag="w1a")
        nc.sync.dma_start(w1a[:], w1.ap()[0:128, :])
        w1c = const.tile([KC, HID], F16, tag="w1c")
        nc.sync.dma_start(w1c[:], w1.ap()[128:185, :])
        w2s = const.tile([128, MT * HID], L2DT, tag="w2s")
        nc.sync.dma_start(w2s[:], w2.ap())
        w3s = const.tile([128, MT], F16, tag="w3s")
        nc.sync.dma_start(w3s[:], w3p.ap())
        if cfg["l3_gpsimd"]:
            w3f = const.tile([128, MT], F32, tag="w3f")
            nc.vector.tensor_copy(w3f[:], w3s[:])
        else:
            w3f = None
        u1s = const.tile([128, MT * UW], UDT, tag="u1s")
        nc.sync.dma_start(u1s[:], u1p.ap())
        u2s = const.tile([128, MT * UW], UDT, tag="u2s")
        nc.sync.dma_start(u2s[:], u2p.ap())
        sc1s = const.tile([1, 1], F32, tag="sc1s")
        nc.sync.dma_start(sc1s[:], sc1.ap())
        sc2s = const.tile([1, 1], F32, tag="sc2s")
        nc.sync.dma_start(sc2s[:], sc2.ap())
        b3s = const.tile([1, 1], F32, tag="b3s")
        nc.sync.dma_start(b3s[:], b3t.ap())
        eps_t = const.tile([1, 1], F32, tag="eps_t")
        nc.vector.memset(eps_t[:], EPS)

        def layer_mms_l2(q_out, m, a1r3, A1):
            """Accumulate layer-2 m-tile into q_out [128, CH]."""
            if cfg["l2_fp8"]:
                w2r3 = w2s[:].rearrange("p (k j) -> p k j", k=MT)
                for t in range(MT // 2):
                    nc.tensor.matmul(
                        q_out, w2r3[:, 2 * t:2 * t + 2, m * 128:(m + 1) * 128],
                        a1r3[:, 2 * t:2 * t + 2, :],
                        start=(t == 0), stop=(t == MT // 2 - 1),
                        perf_mode=PM.DoubleRow)
            else:
                for k in range(MT):
                    nc.tensor.matmul(
                        q_out,
                        w2s[:, k * HID + m * 128: k * HID + (m + 1) * 128],
                        A1[:, k * CH:(k + 1) * CH],
                        start=(k == 0), stop=(k == MT - 1))

        def stats_mms(pst, us, sr3, ts):
            """Emit stats accumulation for k-pair indices ts (subset of the
            group; caller splits the group around other matmuls so PE never
            waits on the last squares)."""
            NT = MT // 2 if cfg["stats_fp8"] else MT
            if cfg["stats_fp8"]:
                us3 = us[:].rearrange("p (k j) -> p k j", k=MT)
                for t in ts:
                    nc.tensor.matmul(pst, us3[:, 2 * t:2 * t + 2, :],
                                     sr3[:, 2 * t:2 * t + 2, :],
                                     start=(t == 0), stop=(t == NT - 1),
                                     perf_mode=PM.DoubleRow,
                                     skip_group_check=True)
            else:
                for t in ts:
                    nc.tensor.matmul(pst, us[:, t:t + 1], sr3[:, t, :],
                                     start=(t == 0), stop=(t == NT - 1),
                                     skip_group_check=True)

        def drain(pool_ps, tag, fill_one, relu_dst, sq_dst, paired,
                  relu_scale=None):
            """Fill PSUM m-tiles via fill_one(psum_slice, m), drain with
            relu (DVE max, optionally scaled per-partition) -> relu_dst and
            square (ScalarE) -> sq_dst.  Each drain op reads PSUM exactly
            once (hardware limit)."""
            if paired:
                for i in range(MT // 2):
                    p = pool_ps.tile([128, 2 * CH], F32, tag=tag)
                    for sub in range(2):
                        fill_one(p[:, sub * CH:(sub + 1) * CH], 2 * i + sub)
                    sl = slice(2 * i * CH, (2 * i + 2) * CH)
                    nc.vector.tensor_scalar(out=relu_dst[:, sl], in0=p[:],
                                            scalar1=0.0, scalar2=None,
                                            op0=ALU.max)
                    nc.scalar.activation(sq_dst[:, sl], p[:], AF.Square)
            else:
                for m in range(MT):
                    p = pool_ps.tile([128, CH], F32, tag=tag)
                    fill_one(p[:], m)
                    sl = slice(m * CH, (m + 1) * CH)
                    if relu_scale is None:
                        nc.vector.tensor_scalar(out=relu_dst[:, sl],
                                                in0=p[:], scalar1=0.0,
                                                scalar2=None, op0=ALU.max)
                    else:
                        nc.vector.tensor_scalar(out=relu_dst[:, sl],
                                                in0=p[:], scalar1=0.0,
                                                scalar2=relu_scale[:, m:m + 1],
                                                op0=ALU.max, op1=ALU.mult)
                    nc.scalar.activation(sq_dst[:, sl], p[:], AF.Square)

        def chunk_l1(c):
            x1 = xin.tile([128, CH], F16, tag="x1")
            nc.sync.dma_start(x1[:], xt.ap()[0:128, c * CH:(c + 1) * CH])
            x2 = xin.tile([KC, CH], F16, tag="x2")
            nc.sync.dma_start(x2[:], xt.ap()[128:185, c * CH:(c + 1) * CH])

            # ---- layer 1: P1' in PSUM -> relu -> A1 (L2DT), square -> S1
            A1 = bigA.tile([128, MT * CH], L2DT, tag="A1")
            S1 = bigS.tile([128, MT * CH], SQDT, tag="S1")

            def fill1(pslice, m):
                nc.tensor.matmul(pslice, w1a[:, m * 128:(m + 1) * 128],
                                 x1[:], start=True, stop=False)
                nc.tensor.matmul(pslice, w1c[:, m * 128:(m + 1) * 128],
                                 x2[:], start=False, stop=True)

            drain(ps_mm, "pmm", fill1, A1, S1, cfg["pair_l1"])
            return A1, S1

        def chunk_rest(c, A1, S1):
            NT = MT // 2 if cfg["stats_fp8"] else MT
            pst1 = ps_st.tile([UW, CH], F32, tag="pst")
            s1r3 = S1[:].rearrange("p (k n) -> p k n", k=MT)
            stats_mms(pst1, u1s, s1r3, range(NT))

            # ---- layer 2: Q2' in PSUM -> relu -> A2 (f16), square -> S2
            A2 = bigA.tile([128, MT * CH], F16, tag="A2")
            S2 = bigS.tile([128, MT * CH], SQDT, tag="S2")
            a1r3 = A1[:].rearrange("p (k n) -> p k n", k=MT)

            def fill2(pslice, m):
                layer_mms_l2(pslice, m, a1r3, A1)

            drain(ps_mm, "pmm", fill2, A2, S2, cfg["pair_l2"],
                  relu_scale=w3f if cfg["l3_gpsimd"] else None)

            pst2 = ps_st.tile([UW, CH], F32, tag="pst")
            s2r3 = S2[:].rearrange("p (k n) -> p k n", k=MT)
            stats_mms(pst2, u2s, s2r3, range(NT))

            # ---- layer 3
            if cfg["l3_gpsimd"]:
                # A2 is already relu(Q2')*w3 per partition; y_raw is a plain
                # sum over k-blocks (GpSimd adds) + partitions (all-reduce),
                # freeing PE of 8 matmuls + 8 weight loads per chunk
                from concourse import bass_isa
                # sequential ping-pong accumulation: each tile's consumer is
                # the next Pool instruction, so 2-buffer rings cannot deadlock
                acc = None
                for k in range(0, MT, 2):
                    tagn = "gsa" if (k // 2) % 2 == 0 else "gsb"
                    t = small.tile([128, CH], F32, tag=tagn,
                                   name=f"gacc_{k}")
                    if acc is None:
                        nc.gpsimd.tensor_add(t[:], A2[:, 0:CH],
                                             A2[:, CH:2 * CH])
                    else:
                        a = small.tile([128, CH], F32, tag="gst",
                                       name=f"gpar_{k}")
                        nc.gpsimd.tensor_add(a[:], A2[:, k * CH:(k + 1) * CH],
                                             A2[:, (k + 1) * CH:(k + 2) * CH])
                        nc.gpsimd.tensor_add(t[:], acc, a[:])
                    acc = t[:]
                yr = small.tile([128, CH], F32, tag="yred")
                nc.gpsimd.partition_all_reduce(yr[:], acc, channels=128,
                                               reduce_op=bass_isa.ReduceOp.add)
                return pst1, pst2, yr
            pst3 = ps_st.tile([1, CH], F32, tag="pst")
            for k in range(MT):
                nc.tensor.matmul(pst3[:], w3s[:, k:k + 1],
                                 A2[:, k * CH:(k + 1) * CH],
                                 start=(k == 0), stop=(k == MT - 1))
            return pst1, pst2, pst3

        def chunk_tail(c, state):
            # division-free LN-scale chain:
            #   e1  = v1 + eps = sc1*pstat1 + eps
            #   D^2 = e1*(v2+eps) = sc2*pstat2 + eps*e1
            #   y   = s1*s2*pst3 + b3 = pst3/D + b3
            # host supplies sc1e = eps*sc1 so e1e = eps*e1 comes in one op
            pst1, pst2, pst3 = state
            e1e = small.tile([1, CH], F32, tag="e1e")
            nc.vector.tensor_scalar(out=e1e[:], in0=pst1[0:1, :],
                                    scalar1=sc1s[:], scalar2=EPS * EPS,
                                    op0=ALU.mult, op1=ALU.add)
            q2s = small.tile([1, CH], F32, tag="q2s")
            nc.vector.tensor_scalar(out=q2s[:], in0=pst2[0:1, :],
                                    scalar1=sc2s[:], scalar2=None,
                                    op0=ALU.mult)
            dd = small.tile([1, CH], F32, tag="dd")
            nc.vector.tensor_add(dd[:], q2s[:], e1e[:])
            D = small.tile([1, CH], F32, tag="D")
            nc.scalar.activation(D[:], dd[:], AF.Sqrt)
            rD = small.tile([1, CH], F32, tag="rD")
            nc.vector.reciprocal(rD[:], D[:])
            yv = small.tile([1, CH], F32, tag="yv")
            nc.vector.tensor_mul(yv[:], rD[:], pst3[0:1, :])
            osb = small.tile([1, CH], F32, tag="osb")
            nc.scalar.activation(osb[:], yv[:], AF.Identity, bias=b3s[:])
            nc.sync.dma_start(y.ap()[0:1, c * CH:(c + 1) * CH], osb[:])

        def probe_chunk(c, A1c, A2c):
            x1 = xin.tile([128, CH], F16, tag="x1")
            nc.sync.dma_start(x1[:], xt.ap()[0:128, c * CH:(c + 1) * CH])
            x2 = xin.tile([KC, CH], F16, tag="x2")
            nc.sync.dma_start(x2[:], xt.ap()[128:185, c * CH:(c + 1) * CH])
            a1r3 = A1c[:].rearrange("p (k n) -> p k n", k=MT)
            for m in range(MT):
                p = ps_mm.tile([128, CH], F32, tag="pmm")
                nc.tensor.matmul(p[:], w1a[:, m * 128:(m + 1) * 128],
                                 x1[:], start=True, stop=False)
                nc.tensor.matmul(p[:], w1c[:, m * 128:(m + 1) * 128],
                                 x2[:], start=False, stop=True)
            if cfg["probe_pairmm"]:
                for m in range(MT):
                    p1 = ps_mm.tile([128, CH], F32, tag="pmm")
                    p2 = ps_mm.tile([128, CH], F32, tag="pmm")
                    for k in range(MT):
                        w = w2s[:, k * HID + m * 128: k * HID + (m + 1) * 128]
                        nc.tensor.matmul(p1[:], w, A1c[:, k * CH:(k + 1) * CH],
                                         start=(k == 0), stop=(k == MT - 1),
                                         skip_group_check=True)
                        nc.tensor.matmul(p2[:], w, A2c[:, k * CH:(k + 1) * CH],
                                         start=(k == 0), stop=(k == MT - 1),
                                         skip_group_check=True)
            else:
                for m in range(MT):
                    p = ps_mm.tile([128, CH], F32, tag="pmm")
                    layer_mms_l2(p[:], m, a1r3, A1c)
            pst3 = ps_st.tile([1, CH], F32, tag="pst")
            for k in range(MT):
                nc.tensor.matmul(pst3[:], w3s[:, k:k + 1],
                                 A2c[:, k * CH:(k + 1) * CH],
                                 start=(k == 0), stop=(k == MT - 1))
            osb = small.tile([1, CH], F32, tag="osb")
            nc.scalar.activation(osb[:], pst3[:], AF.Identity, bias=b3s[:])
            nc.sync.dma_start(y.ap()[0:1, c * CH:(c + 1) * CH], osb[:])

        def pair_l1(ca, cb):
            xs = []
            for c in (ca, cb):
                x1 = xin.tile([128, CH], F16, tag="x1")
                nc.sync.dma_start(x1[:], xt.ap()[0:128, c * CH:(c + 1) * CH])
                x2 = xin.tile([KC, CH], F16, tag="x2")
                nc.sync.dma_start(x2[:], xt.ap()[128:185, c * CH:(c + 1) * CH])
                xs.append((x1, x2))
            outs = []
            for _ in (ca, cb):
                A1 = bigA.tile([128, MT * CH], L2DT, tag="A1")
                S1 = bigS.tile([128, MT * CH], SQDT, tag="S1")
                outs.append((A1, S1))
            for m in range(MT):
                ps = [ps_mm.tile([128, CH], F32, tag="pmm", name=f"pl1_{i}")
                      for i in range(2)]
                for i in range(2):
                    nc.tensor.matmul(ps[i][:], w1a[:, m * 128:(m + 1) * 128],
                                     xs[i][0][:], start=True, stop=False,
                                     skip_group_check=True)
                for i in range(2):
                    nc.tensor.matmul(ps[i][:], w1c[:, m * 128:(m + 1) * 128],
                                     xs[i][1][:], start=False, stop=True,
                                     skip_group_check=True)
                for i in range(2):
                    sl = slice(m * CH, (m + 1) * CH)
                    nc.vector.tensor_scalar(out=outs[i][0][:, sl],
                                            in0=ps[i][:], scalar1=0.0,
                                            scalar2=None, op0=ALU.max)
                    nc.scalar.activation(outs[i][1][:, sl], ps[i][:],
                                         AF.Square)
            return outs

        def pair_stats(us, srs, tag_rows):
            NT = MT // 2 if cfg["stats_fp8"] else MT
            psts = [ps_st.tile([tag_rows, CH], F32, tag="pst", name=f"pstp_{i}")
                    for i in range(2)]
            if cfg["stats_fp8"]:
                us3 = us[:].rearrange("p (k j) -> p k j", k=MT)
                for t in range(NT):
                    for i in range(2):
                        nc.tensor.matmul(psts[i],
                                         us3[:, 2 * t:2 * t + 2, :],
                                         srs[i][:, 2 * t:2 * t + 2, :],
                                         start=(t == 0), stop=(t == NT - 1),
                                         perf_mode=PM.DoubleRow,
                                         skip_group_check=True)
            else:
                for t in range(NT):
                    for i in range(2):
                        nc.tensor.matmul(psts[i], us[:, t:t + 1],
                                         srs[i][:, t, :],
                                         start=(t == 0), stop=(t == NT - 1),
                                         skip_group_check=True)
            return psts

        def pair_rest(a1s1_pair):
            s1rs = [S1[:].rearrange("p (k n) -> p k n", k=MT)
                    for (_, S1) in a1s1_pair]
            pst1s = pair_stats(u1s, s1rs, UW)
            # read the stat banks immediately so the ring keeps flowing
            e1es = []
            for pst1 in pst1s:
                e1e = small.tile([1, CH], F32, tag="e1e")
                nc.vector.tensor_scalar(out=e1e[:], in0=pst1[0:1, :],
                                        scalar1=sc1s[:],
                                        scalar2=EPS * EPS,
                                        op0=ALU.mult, op1=ALU.add)
                e1es.append(e1e)

            a2s2 = []
            for _ in range(2):
                A2 = bigA.tile([128, MT * CH], F16, tag="A2")
                S2 = bigS.tile([128, MT * CH], SQDT, tag="S2")
                a2s2.append((A2, S2))
            a1r3s = [A1[:].rearrange("p (k n) -> p k n", k=MT)
                     for (A1, _) in a1s1_pair]
            for m in range(MT):
                qs = [ps_mm.tile([128, CH], F32, tag="pmm", name=f"pl2_{i}")
                      for i in range(2)]
                if cfg["l2_fp8"]:
                    w2r3 = w2s[:].rearrange("p (k j) -> p k j", k=MT)
                    for t in range(MT // 2):
                        for i in range(2):
                            nc.tensor.matmul(
                                qs[i][:],
                                w2r3[:, 2 * t:2 * t + 2,
                                     m * 128:(m + 1) * 128],
                                a1r3s[i][:, 2 * t:2 * t + 2, :],
                                start=(t == 0), stop=(t == MT // 2 - 1),
                                perf_mode=PM.DoubleRow,
                                skip_group_check=True)
                else:
                    for k in range(MT):
                        w = w2s[:, k * HID + m * 128: k * HID + (m + 1) * 128]
                        for i in range(2):
                            nc.tensor.matmul(
                                qs[i][:], w,
                                a1s1_pair[i][0][:, k * CH:(k + 1) * CH],
                                start=(k == 0), stop=(k == MT - 1),
                                skip_group_check=True)
                for i in range(2):
                    sl = slice(m * CH, (m + 1) * CH)
                    nc.vector.tensor_scalar(out=a2s2[i][0][:, sl],
                                            in0=qs[i][:], scalar1=0.0,
                                            scalar2=None, op0=ALU.max)
                    nc.scalar.activation(a2s2[i][1][:, sl], qs[i][:],
                                         AF.Square)

            s2rs = [S2[:].rearrange("p (k n) -> p k n", k=MT)
                    for (_, S2) in a2s2]
            pst2s = pair_stats(u2s, s2rs, UW)
            q2ss = []
            for pst2 in pst2s:
                q2s = small.tile([1, CH], F32, tag="q2s")
                nc.vector.tensor_scalar(out=q2s[:], in0=pst2[0:1, :],
                                        scalar1=sc2s[:], scalar2=None,
                                        op0=ALU.mult)
                q2ss.append(q2s)

            pst3s = [ps_st.tile([1, CH], F32, tag="pst", name=f"pst3_{i}")
                     for i in range(2)]
            for k in range(MT):
                for i in range(2):
                    nc.tensor.matmul(pst3s[i][:], w3s[:, k:k + 1],
                                     a2s2[i][0][:, k * CH:(k + 1) * CH],
                                     start=(k == 0), stop=(k == MT - 1),
                                     skip_group_check=True)
            return [(e1es[i], q2ss[i], pst3s[i]) for i in range(2)]

        def pair_tail(c, state):
            e1e, q2s, pst3 = state
            dd = small.tile([1, CH], F32, tag="dd")
            nc.vector.tensor_add(dd[:], q2s[:], e1e[:])
            D = small.tile([1, CH], F32, tag="D")
            nc.scalar.activation(D[:], dd[:], AF.Sqrt)
            rD = small.tile([1, CH], F32, tag="rD")
            nc.vector.reciprocal(rD[:], D[:])
            yv = small.tile([1, CH], F32, tag="yv")
            nc.vector.tensor_mul(yv[:], rD[:], pst3[0:1, :])
            osb = small.tile([1, CH], F32, tag="osb")
            nc.scalar.activation(osb[:], yv[:], AF.Identity, bias=b3s[:])
            nc.sync.dma_start(y.ap()[0:1, c * CH:(c + 1) * CH], osb[:])

        def whole_pass_pairs():
            states = None
            for p in range(cfg["nch"] // 2):
                ca, cb = 2 * p, 2 * p + 1
                a1s1 = pair_l1(ca, cb)
                if states is not None:
                    pair_tail(ca - 2, states[0])
                    pair_tail(cb - 2, states[1])
                states = pair_rest(a1s1)
            pair_tail(cfg["nch"] - 2, states[0])
            pair_tail(cfg["nch"] - 1, states[1])

        def whole_pass():
            if cfg["chunk_pairs"]:
                whole_pass_pairs()
                return
            if cfg["probe_mm_only"]:
                A1c = const.tile([128, MT * CH], L2DT, tag="A1c")
                nc.vector.memset(A1c[:], 0.25)
                A2c = const.tile([128, MT * CH], F16, tag="A2c")
                nc.vector.memset(A2c[:], 0.25)
                for c in range(cfg["nch"]):
                    probe_chunk(c, A1c, A2c)
                return
            state = None
            for c in range(cfg["nch"]):
                a1s1 = chunk_l1(c)
                if state is not None:
                    chunk_tail(c - 1, state)
                state = chunk_rest(c, *a1s1)
            chunk_tail(cfg["nch"] - 1, state)

        if loop_iters is None:
            whole_pass()
        else:
            with tc.For_i(0, loop_iters, 1):
                whole_pass()
    nc.compile()
    if cfg["dedupe_ldw"]:
        _dedupe_ldweights(nc)
    return nc


def build_program(simple, loop_iters=None, cfg=None):
    if simple:
        return build_program_fast(loop_iters=loop_iters, cfg=cfg)
    return build_program_general(loop_iters=loop_iters)


# ------------------------------------------------------- general path (slow)
def build_program_general(loop_iters=None):
    """Fallback for nonzero be/bc2 or g != 1: baseline broadcast-LN design,
    correct for arbitrary affine parameters."""
    F32R = F16
    nc = bacc.Bacc("TRN2", target_bir_lowering=False, debug=False)
    xt = nc.dram_tensor("xt", [KA + KC, B_CORE], F32, kind="ExternalInput")
    w1 = nc.dram_tensor("w1", [KA + KC, HID], F32, kind="ExternalInput")
    w2 = nc.dram_tensor("w2", [HID, HID], F32, kind="ExternalInput")
    w3p = nc.dram_tensor("w3p", [128, MT], F32, kind="ExternalInput")
    bc1p = nc.dram_tensor("bc1p", [128, MT], F32, kind="ExternalInput")
    bc2p = nc.dram_tensor("bc2p", [128, MT], F32, kind="ExternalInput")
    g1p = nc.dram_tensor("g1p", [128, MT], F32, kind="ExternalInput")
    be1p = nc.dram_tensor("be1p", [128, MT], F32, kind="ExternalInput")
    g2p = nc.dram_tensor("g2p", [128, MT], F32, kind="ExternalInput")
    be2p = nc.dram_tensor("be2p", [128, MT], F32, kind="ExternalInput")
    b3t = nc.dram_tensor("b3t", [1, 1], F32, kind="ExternalInput")
    y = nc.dram_tensor("y", [1, B_CORE], F32, kind="ExternalOutput")

    from contextlib import ExitStack
    with tile.TileContext(nc) as tc, ExitStack() as ctx, \
            nc.allow_low_precision(reason="f16 rounding is intentional"):
        const = ctx.enter_context(tc.tile_pool(name="const", bufs=1))
        wstage = ctx.enter_context(tc.tile_pool(name="wstage", bufs=2))
        xin = ctx.enter_context(tc.tile_pool(name="xin", bufs=3))
        xr = ctx.enter_context(tc.tile_pool(name="xr", bufs=2))
        bigH = ctx.enter_context(tc.tile_pool(name="bigH", bufs=2))
        bigS = ctx.enter_context(tc.tile_pool(name="bigS", bufs=1))
        bigR1 = ctx.enter_context(tc.tile_pool(name="bigR1", bufs=1))
        bigR2 = ctx.enter_context(tc.tile_pool(name="bigR2", bufs=1))
        small = ctx.enter_context(tc.tile_pool(name="small", bufs=2))
        ps_mm = ctx.enter_context(tc.tile_pool(name="ps_mm", bufs=4, space="PSUM"))
        ps_st = ctx.enter_context(tc.tile_pool(name="ps_st", bufs=2, space="PSUM"))
        ps_vec = ctx.enter_context(tc.tile_pool(name="ps_vec", bufs=2, space="PSUM"))

        w1a_r = const.tile([128, HID], F32R, tag="w1a")
        st = wstage.tile([128, HID], F32, tag="stage")
        nc.sync.dma_start(st[:], w1.ap()[0:128, :])
        nc.vector.tensor_copy(w1a_r[:], st[:])
        w1c_r = const.tile([KC, HID], F32R, tag="w1c")
        stc = wstage.tile([KC, HID], F32, tag="stagec")
        nc.sync.dma_start(stc[:], w1.ap()[128:185, :])
        nc.vector.tensor_copy(w1c_r[:], stc[:])
        w2r = []
        for k in range(MT):
            stk = wstage.tile([128, HID], F32, tag="stage")
            nc.sync.dma_start(stk[:], w2.ap()[k * 128:(k + 1) * 128, :])
            t = const.tile([128, HID], F32R, tag=f"w2r{k}")
            nc.vector.tensor_copy(t[:], stk[:])
            w2r.append(t)
        w3p_r = const.tile([128, MT], F32R, tag="w3p")
        st3 = wstage.tile([128, MT], F32, tag="stages")
        nc.sync.dma_start(st3[:], w3p.ap())
        nc.vector.tensor_copy(w3p_r[:], st3[:])

        def load_small(name, dram):
            t = const.tile([128, MT], F32, tag=name)
            nc.sync.dma_start(t[:], dram.ap())
            return t
        bc1s = load_small("bc1s", bc1p); bc2s = load_small("bc2s", bc2p)
        g1s = load_small("g1s", g1p); be1s = load_small("be1s", be1p)
        g2s = load_small("g2s", g2p); be2s = load_small("be2s", be2p)
        b3s = const.tile([1, 1], F32, tag="b3s")
        nc.sync.dma_start(b3s[:], b3t.ap())
        ones_st = const.tile([128, 1], F32, tag="ones_st")
        nc.vector.memset(ones_st[:], 1.0)
        ones_col = const.tile([128, 1], F32R, tag="ones_col")
        nc.vector.tensor_copy(ones_col[:], ones_st[:])
        ones_rst = const.tile([1, 128], F32, tag="ones_rst")
        nc.vector.memset(ones_rst[:], 1.0)
        ones_row = const.tile([1, 128], F32R, tag="ones_row")
        nc.vector.tensor_copy(ones_row[:], ones_rst[:])
        eps_t = const.tile([1, 1], F32, tag="eps_t")
        nc.vector.memset(eps_t[:], EPS)

        def layer_norm_relu(Hb, g_s, be_s, out_pool, out_tag):
            sqb = bigS.tile([128, MT * CH], F32R, tag="sq")
            for m in range(MT):
                sl = slice(m * CH, (m + 1) * CH)
                nc.vector.tensor_mul(sqb[:, sl], Hb[:, sl], Hb[:, sl])
            pst = ps_st.tile([1, CH], F32, tag="pst")
            for m in range(MT):
                nc.tensor.matmul(pst[:], ones_col[:],
                                 sqb[:, m * CH:(m + 1) * CH],
                                 start=(m == 0), stop=(m == MT - 1))
            sd = small.tile([1, CH], F32, tag="sd")
            nc.scalar.activation(sd[:], pst[:], AF.Sqrt,
                                 bias=eps_t[:], scale=1.0 / HID)
            rs = small.tile([1, CH], F32R, tag="rs")
            nc.vector.reciprocal(rs[:], sd[:])
            pv = ps_vec.tile([128, CH], F32, tag="pv")
            nc.tensor.matmul(pv[:], ones_row[:], rs[:], start=True, stop=True)
            Rb = out_pool.tile([128, MT * CH], F32R, tag=out_tag)
            for m in range(MT):
                sl = slice(m * CH, (m + 1) * CH)
                nc.vector.tensor_mul(Hb[:, sl], Hb[:, sl], pv[:])
                nc.scalar.activation(Rb[:, sl], Hb[:, sl], AF.Relu,
                                     bias=be_s[:, m:m + 1],
                                     scale=g_s[:, m:m + 1])
            return Rb

        def chunk_body(c):
            x1 = xin.tile([128, CH], F32, tag="x1")
            nc.sync.dma_start(x1[:], xt.ap()[0:128, c * CH:(c + 1) * CH])
            x2 = xin.tile([KC, CH], F32, tag="x2")
            nc.sync.dma_start(x2[:], xt.ap()[128:185, c * CH:(c + 1) * CH])
            x1r = xr.tile([128, CH], F32R, tag="x1r")
            nc.vector.tensor_copy(x1r[:], x1[:])
            xab = xr.tile([KC, CH], F32, tag="xab")
            nc.vector.tensor_scalar(
                out=xab[:].bitcast(mybir.dt.int32),
                in0=x2[:].bitcast(mybir.dt.int32),
                scalar1=0x7FFFFFFF, scalar2=None, op0=ALU.bitwise_and)
            xln = xr.tile([KC, CH], F32, tag="xln")
            nc.scalar.activation(xln[:], xab[:], AF.Ln, bias=1.0)
            xsg = xr.tile([KC, CH], F32, tag="xsg")
            nc.scalar.activation(xsg[:], x2[:], AF.Sign)
            x2r = xr.tile([KC, CH], F32R, tag="x2r")
            nc.vector.tensor_mul(x2r[:], xsg[:], xln[:])

            H1 = bigH.tile([128, MT * CH], F32, tag="H")
            for m in range(MT):
                p1 = ps_mm.tile([128, CH], F32, tag="pmm")
                nc.tensor.matmul(p1[:], w1a_r[:, m * 128:(m + 1) * 128],
                                 x1r[:], start=True, stop=False)
                nc.tensor.matmul(p1[:], w1c_r[:, m * 128:(m + 1) * 128],
                                 x2r[:], start=False, stop=True)
                nc.scalar.activation(H1[:, m * CH:(m + 1) * CH], p1[:],
                                     AF.Identity, bias=bc1s[:, m:m + 1])
            R1 = layer_norm_relu(H1, g1s, be1s, bigR1, "R1")

            H2 = bigH.tile([128, MT * CH], F32, tag="H")
            for m in range(MT):
                p2 = ps_mm.tile([128, CH], F32, tag="pmm")
                for k in range(MT):
                    nc.tensor.matmul(p2[:], w2r[k][:, m * 128:(m + 1) * 128],
                                     R1[:, k * CH:(k + 1) * CH],
                                     start=(k == 0), stop=(k == MT - 1))
                nc.scalar.activation(H2[:, m * CH:(m + 1) * CH], p2[:],
                                     AF.Identity, bias=bc2s[:, m:m + 1])
            R2 = layer_norm_relu(H2, g2s, be2s, bigR2, "R2")

            p3 = ps_st.tile([1, CH], F32, tag="pst")
            for k in range(MT):
                nc.tensor.matmul(p3[:], w3p_r[:, k:k + 1],
                                 R2[:, k * CH:(k + 1) * CH],
                                 start=(k == 0), stop=(k == MT - 1))
            osb = small.tile([1, CH], F32, tag="osb")
            nc.scalar.activation(osb[:], p3[:], AF.Identity, bias=b3s[:])
            nc.sync.dma_start(y.ap()[0:1, c * CH:(c + 1) * CH], osb[:])

        if loop_iters is None:
            for c in range(NCH):
                chunk_body(c)
        else:
            with tc.For_i(0, loop_iters, 1):
                for c in range(NCH):
                    chunk_body(c)
    nc.compile()
    return nc


# ---------------------------------------------------------------- entry point
_CACHE = {}

# Measured-best on HW (back-to-back min-of-18): 426.5us/pass with layer-3
# on GpSimd vs 442.2us without.  The chunk-pair + LDW-dedupe mode removes
# 32 weight loads per chunk but its schedule measured slower; kept
# available but off.
BEST_CFG = dict(DEFAULT_CFG, pair_l1=False, pair_l2=False, ps_mm_bufs=5,
                l3_gpsimd=True)


def _get_program(simple):
    key = ("prog", simple)
    if key not in _CACHE:
        _CACHE[key] = build_program(simple, cfg=BEST_CFG if simple else None)
    return _CACHE[key]


def _make_fast_maps(inp, W1c, bc1, W2c, cfg):
    H = HID
    W1x = W1c.copy()
    W1x[125] = bc1                      # bias via constant-1 feature row
    XT = _build_xt(inp, bias_row=True, host_ln=True)

    # per-hidden scales from a strided data sample
    idx = np.arange(0, XT.shape[1], max(1, XT.shape[1] // 512))[:512]
    Xs = XT[:, idx].astype(np.float64)
    P1s = W1x.T @ Xs                    # [H, S]
    sd1 = P1s.std(axis=1) + 1e-9
    c1 = T1 / sd1
    A1s = np.maximum(P1s, 0.0)
    Q2s = W2c.T @ A1s
    sd2 = Q2s.std(axis=1) + 1e-9
    c2 = T2 / sd2

    W1s = _to_f16(W1x * c1[None, :])
    W2f = (W2c / c1[:, None]) * c2[None, :]          # [H, H]
    # pack [p, k*HID] with row h = k*128+p
    W2p = W2f.reshape(MT, 128, H).transpose(1, 0, 2).reshape(128, MT * H)
    w2dev = _to_e4(W2p) if cfg["l2_fp8"] else _to_f16(W2p)

    u1 = 1.0 / (H * c1 * c1)
    k1 = int(np.floor(np.log2(160.0 / u1.max())))
    u2 = 1.0 / (H * c2 * c2)
    k2 = int(np.floor(np.log2(160.0 / u2.max())))
    W3 = np.asarray(inp["W3"], np.float64)[:, 0]
    b3 = np.asarray(inp["b3"], np.float64)

    def upack(u):
        # [1024] -> [128, MT*UW]; value at [p, k*UW], zeros elsewhere
        if not cfg["stats_fp8"]:
            return _pack128(u, np.float16)
        UW = 32
        out = np.zeros((128, MT * UW), np.float64)
        out[:, ::UW] = _pack128(u, np.float64)
        return _to_e4(out)

    shared = {
        "w1": np.ascontiguousarray(W1s),
        "w2": np.ascontiguousarray(w2dev),
        "w3p": _pack128(W3 / c2, np.float16),
        "u1p": upack(u1 * 2.0 ** k1),
        "u2p": upack(u2 * 2.0 ** k2),
        "sc1": np.full((1, 1), EPS * 2.0 ** -k1, np.float32),
        "sc2": np.full((1, 1), 2.0 ** -k2, np.float32),
        "b3t": np.asarray(b3, np.float32).reshape(1, 1),
    }
    XT16 = _to_f16(XT)
    in_maps = []
    for c in range(N_CORES):
        m = dict(shared)
        m["xt"] = np.ascontiguousarray(XT16[:, c * B_CORE:(c + 1) * B_CORE])
        in_maps.append(m)
    return in_maps


def _make_general_maps(inp, W1c, bc1, W2c, bc2):
    XT = _build_xt(inp, bias_row=False, host_ln=False)
    g1 = np.asarray(inp["g1"], np.float32); be1 = np.asarray(inp["be1"], np.float32)
    g2 = np.asarray(inp["g2"], np.float32); be2 = np.asarray(inp["be2"], np.float32)
    W3 = np.asarray(inp["W3"], np.float32)
    b3 = np.asarray(inp["b3"], np.float32)
    shared = {
        "w1": W1c.astype(np.float32), "w2": W2c.astype(np.float32),
        "w3p": _pack128(W3[:, 0]),
        "bc1p": _pack128(bc1), "bc2p": _pack128(bc2),
        "g1p": _pack128(g1), "be1p": _pack128(be1),
        "g2p": _pack128(g2), "be2p": _pack128(be2),
        "b3t": b3.reshape(1, 1),
    }
    in_maps = []
    for c in range(N_CORES):
        m = dict(shared)
        m["xt"] = np.ascontiguousarray(
            XT[:, c * B_CORE:(c + 1) * B_CORE].astype(np.float32))
        in_maps.append(m)
    return in_maps


def make_in_maps(inputs, cfg=None):
    cfg = {**DEFAULT_CFG, **(cfg or BEST_CFG)}
    inp = {k: np.asarray(v) for k, v in inputs.items()}
    W1c, bc1, W2c, bc2 = _fold_weights(inp)
    g1 = np.asarray(inp["g1"]); be1 = np.asarray(inp["be1"])
    g2 = np.asarray(inp["g2"]); be2 = np.asarray(inp["be2"])
    simple = bool(
        np.all(g1 == 1.0) and np.all(g2 == 1.0)
        and np.all(be1 == 0.0) and np.all(be2 == 0.0)
        and np.all(np.abs(bc2) < 1e-12))
    if simple:
        return _make_fast_maps(inp, W1c, bc1, W2c, cfg), True
    return _make_general_maps(inp, W1c, bc1, W2c, bc2), False


def kernel(**inputs) -> np.ndarray:
    in_maps, simple = make_in_maps(inputs)
    nc = _get_program(simple)
    res = run_bass_kernel_spmd(nc, in_maps, core_ids=list(range(N_CORES)))
    y = np.concatenate([r["y"][0] for r in res.results])
    return y.reshape(B, 1).astype(np.float32)


if __name__ == "__main__":
    import jax
    import reference
    cpu = jax.devices("cpu")[0]
    with jax.default_device(cpu):
        inp = reference.setup_inputs()
        ref = np.asarray(reference.reference(**inp))
    out = kernel(**{k: np.asarray(v) for k, v in inp.items()})
    err = np.abs(out - ref)
    scale = np.abs(ref).max()
    print("max_abs", err.max(), "rel(vs scale)", err.max() / scale,
          "mean_rel", (err / (np.abs(ref) + 1e-6)).mean())
